# revision 67
# baseline (speedup 1.0000x reference)
"""Trainium2 Bass kernel for nn_BottleneckResAtnMHSA (8 NeuronCores, SPMD).

Reference computation (per image, C=128, N=1024 spatial tokens):
  x1 = silu(bn1(w1 @ x))                      # [128, 1024]
  q/k/v = w{q,k,v} @ x1 + b{q,k,v}            # [128, 1024]
  logits = q^T k + pos^T q                    # [1024, 1024]
  att = softmax(logits, axis=-1)
  out = v @ att^T                             # [128, 1024]
  y = x + silu(bn2(w2 @ out))                 # [256, 1024]

Sharding: data-parallel over batch, 4 images per core, params replicated.

Kernel design (v4):
  * BN scales folded into conv weights host-side; q/k biases fold into the
    precomputed position matrix; v's bias folds into cv2's bias.
  * q and k are never materialized: attT = x1^T (M x1 + ppu) with
    M = Wk^T Wq and ppu precomputed host-side (i-only terms cancel in
    softmax), so attention is a single K=128 matmul layer.
  * Everything post-cv1 runs in bf16 (x1, G, vT, e, outn, params): PE rate
    is unchanged (1 cyc/row), DVE element ops get the 2x 16-bit mode, and
    SBUF footprints halve.  cv1 itself is float32r on the fp32 x.
  * Softmax runs without max subtraction (|logits| < 40 for this model's
    data distribution); normalization is reciprocal_approx_fast on the
    denominator (~18 correct bits, one plain-rate DVE op) plus one
    multiply -- the walrus backend has no DVE divide.
  * Softmax denominators: two levels of DVE pair-sums (bf16, 2x mode)
    reduce the 8 exp tiles to 2; a transient all-ones [128,128] stationary
    matmul column-sums those, broadcasting across partitions.
  * PSUM: one [128,1024] two-bank pool (bufs=3) rotates cv1/G/vT/att/cv2/s
    tiles; only the o-accumulator owns banks full-time.  Each attention
    block is exp'd by ONE wide ACT instruction (8/image).
  * The whole kernel is ONE software pipeline: the steady-state window is
    image k's 8-block attention loop, and other images' preprocessing /
    cv2 epilogues / x-DMAs are emitted INTO fixed slots of that loop so
    every engine's strict-FIFO queue sees work in globally-useful order.
    Each image's denominator matmul + normalize runs at the head of the
    NEXT window so it never blocks new att blocks in PE's in-order queue.
  * Silu-LUT work is two-window batched ('batch2'): windows b1/b3 carry
    all silu activations (cv1 of images 2&3, cv2 of the finished images)
    as one contiguous ACT run whose input psums are slotted so no exp can
    interleave; windows b0/b2 are pure-exp.  Total Silu<->Exp LUT traffic
    is 7 loads (the baseline paid 9 + scattered stalls; loads are 1.28us).
  * The slow-but-PSUM-blind GpSimd engine absorbs the SBUF-only residual
    adds; vT's psum->sbuf copy rides DVE.  The wall-clock-tail image gets
    dedicated y/res tiles, half-granular epilogue chains, and an early
    LUT warm so the final normalize->cv2->store drains fast.

  TimelineSim: 80769 ns vs the 88621 ns baseline; rel err 5.2e-03.
"""

import numpy as np

N_CORES = 8
B_PER_CORE = 4
C = 128
CIN = 256
N = 1024
EPS = 1e-5

_CACHE = {}

# scheduling knobs, tuned via TimelineSim sweeps
EBUFS = 1
EPBUFS = 1

CFG = {
    'res': 'pool3',      # residual add engine: dve|pool3|pool_all
    'sched': 'batch2',   # silu batching: per_window | batch2
    'pairs': 'l2',       # denominator pair-sum depth: l1|l2
}


def _build_program(cfg=None):
    import concourse.bacc as bacc
    import concourse.mybir as mybir
    import concourse.tile as tile

    cfg = {**CFG, **(cfg or {})}

    f32 = mybir.dt.float32
    bf16 = mybir.dt.bfloat16
    AF = mybir.ActivationFunctionType
    ALU = mybir.AluOpType

    nc = bacc.Bacc("TRN2", target_bir_lowering=False, debug=False,
                   num_devices=N_CORES)

    xs = nc.dram_tensor("xs", [B_PER_CORE, CIN, N], f32, kind="ExternalInput").ap()
    w1t = nc.dram_tensor("w1t", [CIN, C], f32, kind="ExternalInput").ap()
    mqk = nc.dram_tensor("mqk", [C, C], bf16, kind="ExternalInput").ap()
    wvt = nc.dram_tensor("wvt", [C, C], bf16, kind="ExternalInput").ap()
    ppu = nc.dram_tensor("ppu", [C, N], bf16, kind="ExternalInput").ap()
    w2t = nc.dram_tensor("w2t", [C, CIN], bf16, kind="ExternalInput").ap()
    bpk = nc.dram_tensor("bpk", [C, 3], f32, kind="ExternalInput").ap()
    onesd = nc.dram_tensor("onesd", [C, C], bf16, kind="ExternalInput").ap()
    ys = nc.dram_tensor("ys", [B_PER_CORE, CIN, N], f32, kind="ExternalOutput").ap()

    HALF = [slice(0, 512), slice(512, 1024)]
    f32r = mybir.dt.float32r
    fr = lambda ap: ap.bitcast(f32r)

    with tile.TileContext(nc) as tc:
        with (
            tc.tile_pool(name="consts", bufs=1) as consts,
            tc.tile_pool(name="act", bufs=1) as act,
            tc.tile_pool(name="psum", bufs=1, space="PSUM") as psum,
            nc.allow_low_precision(reason="bf16 pipeline validated vs fp64 reference"),
        ):
            bpk_sb = consts.tile([128, 3], f32, tag="bpk")
            w1t_sb = consts.tile([128, 2, C], f32, tag="w1t")
            mqk_sb = consts.tile([128, C], bf16, tag="mqk")
            wvt_sb = consts.tile([128, C], bf16, tag="wvt")
            ppu_sb = consts.tile([128, N], bf16, tag="ppu")
            w2t_sb = consts.tile([128, CIN], bf16, tag="w2t")
            t1c = bpk_sb[:, 0:1]
            b2c = [bpk_sb[:, 1:2], bpk_sb[:, 2:3]]

            ones_sb = consts.tile([128, 128], bf16, tag="ones")
            nc.sync.dma_start(ones_sb, onesd)
            # warm the silu LUT at t~0 so image 0's cv1 isn't stuck behind
            # a late table load
            warm_sb = consts.tile([128, 1], f32, tag="warm")
            nc.vector.memset(warm_sb, 0.0)
            nc.scalar.activation(warm_sb, warm_sb, AF.Silu, scale=0.0)

            def mm(out, lhsT, rhs, **kw):
                nc.tensor.matmul(out, lhsT, rhs, **kw)

            x_sbs, x1_sbs, g_sbs, vt_sbs, outn_sbs = {}, {}, {}, {}, {}
            cv1_ps, cv2_ps = {}, {}

            def dma_x(b):
                if b >= B_PER_CORE or b in x_sbs:
                    return
                x_sb = act.tile([128, 2, N], f32, tag="x", bufs=4, name=f"x{b}")
                xr = xs[b].rearrange("(k p) n -> p k n", p=128)
                if b == 0:
                    # need-ordered cold start: first half's two k-quarters,
                    # then cv1 weights + biases, then the second half
                    nc.sync.dma_start(fr(x_sb[:, 0, HALF[0]]), fr(xr[:, 0, HALF[0]]))
                    nc.sync.dma_start(
                        fr(w1t_sb), fr(w1t.rearrange("(k p) m -> p k m", p=128)))
                    nc.sync.dma_start(bpk_sb, bpk)
                    nc.sync.dma_start(fr(x_sb[:, 1, HALF[0]]), fr(xr[:, 1, HALF[0]]))
                    nc.sync.dma_start(fr(x_sb[:, 0, HALF[1]]), fr(xr[:, 0, HALF[1]]))
                    nc.sync.dma_start(fr(x_sb[:, 1, HALF[1]]), fr(xr[:, 1, HALF[1]]))
                else:
                    for k in range(2):
                        for h in HALF:
                            nc.sync.dma_start(fr(x_sb[:, k, h]), fr(xr[:, k, h]))
                x_sbs[b] = x_sb

            def cv1mm(b):
                dma_x(b)
                x_sb = x_sbs[b]
                ps = psum.tile([128, N], f32, tag="mm", bufs=3, name=f"psx1_{b}")
                for hi, h in enumerate(HALF):
                    for k in range(2):
                        mm(ps[:, h], fr(w1t_sb[:, k, :]), fr(x_sb[:, k, h]),
                           start=(k == 0), stop=(k == 1))
                cv1_ps[b] = ps
                if b == 0:
                    # params ride the DMA queue right behind x(0)
                    nc.sync.dma_start(mqk_sb, mqk)
                    nc.sync.dma_start(ppu_sb, ppu)
                    nc.sync.dma_start(wvt_sb, wvt)

            def u1_silu(b):
                x1_sb = act.tile([128, N], bf16, tag="x1", bufs=3, name=f"x1_{b}")
                if b == 0:
                    # per-half: x1/G/att h0 chains start before x h1 lands
                    for h in HALF:
                        nc.scalar.activation(x1_sb[:, h], cv1_ps[b][:, h],
                                             AF.Silu, bias=t1c)
                else:
                    nc.scalar.activation(x1_sb, cv1_ps[b], AF.Silu, bias=t1c)
                x1_sbs[b] = x1_sb

            def g_chunk(b):
                # G = (Wk^T Wq) @ x1 + (Wq^T pos + (Wk^T bq) 1^T); logits
                # are then attT = x1^T G -- q and k never materialize.
                x1_sb = x1_sbs[b]
                g_sb = act.tile([128, N], bf16, tag="g", bufs=2, name=f"g{b}")
                psg = psum.tile([128, N], f32, tag="mm", bufs=3, name=f"psg_{b}")
                for h in HALF:
                    mm(psg[:, h], mqk_sb, x1_sb[:, h], start=True, stop=True)
                    nc.vector.tensor_add(g_sb[:, h], psg[:, h], ppu_sb[:, h])
                g_sbs[b] = g_sb
                if b == 0:
                    nc.sync.dma_start(w2t_sb, w2t)

            def vt_chunk(b):
                # vT: 8 [128j,128c] blocks copied out as one [128,1024] op
                x1_sb = x1_sbs[b]
                vt_sb = act.tile([128, N], bf16, tag="vt", bufs=2, name=f"vt{b}")
                psv = psum.tile([128, 8, C], f32, tag="mm", bufs=3,
                                name=f"psvt_{b}")
                for jt in range(8):
                    sl = slice(jt * 128, (jt + 1) * 128)
                    mm(psv[:, jt, :], x1_sb[:, sl], wvt_sb,
                       start=True, stop=True)
                nc.vector.tensor_copy(
                    vt_sb.rearrange("p (a c) -> p a c", a=8), psv)
                vt_sbs[b] = vt_sb

            def cv2mm(b, mt):
                ps = psum.tile([128, N], f32, tag="mm", bufs=3,
                               name=f"psy_{b}_{mt}")
                for h in HALF:
                    mm(ps[:, h], w2t_sb[:, mt * 128:(mt + 1) * 128],
                       outn_sbs[b][:, h], start=True, stop=True)
                cv2_ps[(b, mt)] = ps

            def cv2fin(b, mt):
                # y = silu(w2p @ outn + b2); out = x + y.  The tail image
                # gets its own tags so it never WARs against the slow Pool
                # residual queue of earlier images.
                ytag, rtag = ("y3", "res3") if b == 3 else ("y", "res")
                y_sb = act.tile([128, N], bf16, tag=ytag, bufs=4,
                                name=f"y{b}_{mt}")
                res_sb = act.tile([128, N], f32, tag=rtag, bufs=4,
                                  name=f"res{b}_{mt}")
                if b == 3:
                    # wall-clock tail: half-granular so res/DMA start early
                    for h in HALF:
                        nc.scalar.activation(y_sb[:, h], cv2_ps[(b, mt)][:, h],
                                             AF.Silu, bias=b2c[mt])
                else:
                    nc.scalar.activation(y_sb, cv2_ps[(b, mt)], AF.Silu,
                                         bias=b2c[mt])
                for hi, h in enumerate(HALF):
                    # the residual add is SBUF-only, so the slow-but-idle
                    # GpSimd absorbs it for all but the wall-clock-tail
                    # image, whose halves split across DVE+GpSimd so the
                    # two chains drain in parallel
                    if cfg['res'] == 'mix3' and b == 3:
                        add_eng = nc.vector if hi == 0 else nc.gpsimd
                    elif cfg['res'] == 'dve' or \
                            (cfg['res'] in ('pool3', 'mix3') and b == 3):
                        add_eng = nc.vector
                    else:
                        add_eng = nc.gpsimd
                    add_eng.tensor_add(res_sb[:, h], y_sb[:, h],
                                       x_sbs[b][:, mt, h])
                    nc.sync.dma_start(ys[b, mt * 128:(mt + 1) * 128, h],
                                      res_sb[:, h])

            ep2_sbs = {}
            o_psums = {}

            def finish_b(b):
                # transient denominator: ones^T (ep2_0 | ep2_1), broadcast
                # across partitions; normalize with approx-reciprocal (~18
                # bits) + multiply.  Runs at the head of the NEXT window so
                # it never blocks the next image's att blocks in PE's
                # in-order queue.
                src_sb, nq = ep2_sbs[b]
                ps_s = psum.tile([128, N], f32, tag="mm", bufs=3,
                                 name=f"pss_{b}")
                # h-outer so the tail's h0 normalize chain starts earliest
                for h in HALF:
                    for q in range(nq):
                        mm(ps_s[:, h], ones_sb, src_sb[:, q, h],
                           start=(q == 0), stop=(q == nq - 1),
                           skip_group_check=True)
                r_sb = act.tile([128, N], f32, tag="recip", bufs=2,
                                name=f"r{b}")
                outn_sb = act.tile([128, N], bf16, tag="outn", bufs=3,
                                   name=f"outn{b}")
                if b == 3:
                    # tail: half-granular so cv2's first matmul starts early
                    for h in HALF:
                        nc.vector.reciprocal_approx_fast(r_sb[:, h],
                                                         ps_s[:, h])
                        nc.vector.tensor_mul(outn_sb[:, h], o_psums[b][:, h],
                                             r_sb[:, h])
                else:
                    nc.vector.reciprocal_approx_fast(r_sb, ps_s)
                    for h in HALF:
                        nc.vector.tensor_mul(outn_sb[:, h], o_psums[b][:, h],
                                             r_sb[:, h])
                outn_sbs[b] = outn_sb

            def phase_b(b, pre, slots):
                # image b's attention loop; filler chunks (other images'
                # prep / epilogue / DMA) drop into fixed slots
                x1_sb, g_sb, vt_sb = x1_sbs[b], g_sbs[b], vt_sbs[b]
                ps_o = psum.tile([128, N], f32, tag="o", bufs=1, name=f"pso_{b}")
                e_sb = act.tile([128, 8, N], bf16, tag="e", bufs=EBUFS, name=f"e{b}")
                ep1_sb = act.tile([128, 4, N], bf16, tag="ep1", bufs=EPBUFS,
                                  name=f"ep1{b}")
                ep2_sb = act.tile([128, 2, N], bf16, tag="ep2", bufs=1,
                                  name=f"ep2{b}")

                def emit_att(jt):
                    sl = slice(jt * 128, (jt + 1) * 128)
                    ps = psum.tile([128, N], f32, tag="mm", bufs=3,
                                   name=f"psatt_{b}_{jt}")
                    for h in HALF:
                        mm(ps[:, h], x1_sb[:, sl], g_sb[:, h],
                           start=True, stop=True)
                    return ps

                def emit_o(jt):
                    sl = slice(jt * 128, (jt + 1) * 128)
                    for h in HALF:
                        # numerator: out[c, i] += sum_j v[c, j] e[j, i]
                        mm(ps_o[:, h], vt_sb[:, sl], e_sb[:, jt, h],
                           start=(jt == 0), stop=(jt == 7),
                           skip_group_check=True)

                for f in pre:
                    f()
                atts = {jt: emit_att(jt) for jt in range(3)}
                for jt in range(8):
                    cur = atts.pop(jt)
                    nc.scalar.activation(e_sb[:, jt, :], cur, AF.Exp)
                    if jt + 3 <= 7:
                        atts[jt + 3] = emit_att(jt + 3)
                    if jt >= 1:
                        emit_o(jt - 1)
                    if jt % 2 == 1:
                        p = jt // 2
                        l2 = cfg['pairs'] == 'l2'
                        if b == B_PER_CORE - 1 and jt == 7:
                            # tail: half-granular pair tree so the h0
                            # denominator matmul fires earliest
                            for h in HALF:
                                nc.vector.tensor_add(ep1_sb[:, p, h],
                                                     e_sb[:, jt - 1, h],
                                                     e_sb[:, jt, h])
                                if l2:
                                    nc.vector.tensor_add(ep2_sb[:, p // 2, h],
                                                         ep1_sb[:, p - 1, h],
                                                         ep1_sb[:, p, h])
                            continue
                        nc.vector.tensor_add(ep1_sb[:, p, :],
                                             e_sb[:, jt - 1, :],
                                             e_sb[:, jt, :])
                        if l2 and p % 2 == 1:
                            nc.vector.tensor_add(ep2_sb[:, p // 2, :],
                                                 ep1_sb[:, p - 1, :],
                                                 ep1_sb[:, p, :])
                    for f in slots.get(jt, []):
                        f()
                emit_o(7)
                o_psums[b] = ps_o
                ep2_sbs[b] = (ep2_sb, 2) if cfg['pairs'] == 'l2' \
                    else (ep1_sb, 4)

            F = lambda fn, *a: (lambda: fn(*a))

            def silu_batch(*tasks):
                def run():
                    for t in tasks:
                        t()
                return run

            if cfg['sched'] == 'per_window':
                # one silu batch per window: u1(b+1) + cv2 of b-1
                cv1mm(0)
                u1_silu(0)
                g_chunk(0)
                vt_chunk(0)
                dma_x(1)
                for b in range(B_PER_CORE):
                    prv, nxt = b - 1, b + 1
                    pre = [F(finish_b, prv)] if prv >= 0 else []
                    slots = {}
                    if nxt < B_PER_CORE:
                        slots[0] = [F(cv1mm, nxt)]
                    if prv >= 0:
                        slots[1] = [F(cv2mm, prv, 0)]
                        slots[2] = [F(cv2mm, prv, 1)]
                    batch = []
                    if nxt < B_PER_CORE:
                        batch.append(F(u1_silu, nxt))
                    if prv >= 0:
                        batch += [F(cv2fin, prv, 0), F(cv2fin, prv, 1)]
                    slots[4] = [silu_batch(*batch)] if batch else []
                    if nxt < B_PER_CORE:
                        slots[5] = [F(g_chunk, nxt)]
                        slots[6] = [F(vt_chunk, nxt), F(dma_x, b + 2)]
                    phase_b(b, pre, slots)
                # pull the tail's silu table load under fin(3)'s DVE work
                nc.scalar.activation(warm_sb, warm_sb, AF.Silu, scale=0.0)
                finish_b(3)
                cv2mm(3, 0)
                cv2fin(3, 0)
                cv2mm(3, 1)
                cv2fin(3, 1)
            else:
                # batch2: silu work rides only windows b1/b3 (and the ends),
                # so b0/b2 run pure-exp with zero LUT swaps
                cv1mm(0)
                u1_silu(0)
                dma_x(1)
                cv1mm(1)
                u1_silu(1)
                g_chunk(0)
                vt_chunk(0)
                phase_b(0, [], {
                    0: [F(dma_x, 2)],
                    1: [F(g_chunk, 1)],
                    2: [F(vt_chunk, 1)],
                    5: [F(dma_x, 3)],
                })
                phase_b(1, [F(finish_b, 0)], {
                    0: [F(cv1mm, 2)],
                    1: [F(cv1mm, 3)],
                    2: [F(cv2mm, 0, 0), F(cv2mm, 0, 1)],
                    4: [silu_batch(F(u1_silu, 2), F(u1_silu, 3),
                                   F(cv2fin, 0, 0), F(cv2fin, 0, 1))],
                    5: [F(g_chunk, 2)],
                    6: [F(vt_chunk, 2)],
                })
                phase_b(2, [F(finish_b, 1)], {
                    0: [F(g_chunk, 3)],
                    1: [F(vt_chunk, 3)],
                })
                phase_b(3, [F(finish_b, 2)], {
                    0: [F(cv2mm, 1, 0)],
                    1: [F(cv2mm, 1, 1)],
                    2: [F(cv2mm, 2, 0), F(cv2mm, 2, 1)],
                    4: [silu_batch(F(cv2fin, 1, 0), F(cv2fin, 1, 1),
                                   F(cv2fin, 2, 0), F(cv2fin, 2, 1))],
                })
                # pull the tail's silu table load under fin(3)'s DVE work
                nc.scalar.activation(warm_sb, warm_sb, AF.Silu, scale=0.0)
                finish_b(3)
                cv2mm(3, 0)
                cv2fin(3, 0)
                cv2mm(3, 1)
                cv2fin(3, 1)

    nc.compile()
    return nc


def _prepare_params(w1, bn1_g, bn1_b, bn1_m, bn1_v, wq, bq, wk, bk, wv, bv,
                    rel_h, rel_w, w2, bn2_g, bn2_b, bn2_m, bn2_v):
    import ml_dtypes
    f64 = np.float64
    bf16 = ml_dtypes.bfloat16
    s1 = bn1_g.astype(f64) / np.sqrt(bn1_v.astype(f64) + EPS)
    w1p = w1.astype(f64) * s1[:, None]
    t1 = bn1_b.astype(f64) - bn1_m.astype(f64) * s1
    s2 = bn2_g.astype(f64) / np.sqrt(bn2_v.astype(f64) + EPS)
    w2p = w2.astype(f64) * s2[:, None]
    t2 = bn2_b.astype(f64) - bn2_m.astype(f64) * s2
    b2 = t2 + w2p @ bv.astype(f64)
    posv = (rel_h.astype(f64) + rel_w.astype(f64)).reshape(C, N)
    f32 = np.float32
    bpk = np.stack([t1, b2[:128], b2[128:]], axis=1)
    mqk_m = wq.astype(f64).T @ wk.astype(f64)
    ppu_m = wq.astype(f64).T @ posv + (wk.astype(f64).T @ bq.astype(f64))[:, None]
    return {
        "w1t": np.ascontiguousarray(w1p.T, dtype=f32),
        "mqk": np.ascontiguousarray(mqk_m, dtype=f64).astype(bf16),
        "wvt": np.ascontiguousarray(wv.T).astype(bf16),
        "ppu": np.ascontiguousarray(ppu_m, dtype=f64).astype(bf16),
        "w2t": np.ascontiguousarray(w2p.T, dtype=f64).astype(bf16),
        "bpk": np.ascontiguousarray(bpk, dtype=f32),
        "onesd": np.ones((C, C), dtype=bf16),
    }


def get_program(cfg=None):
    key = ("nc",) + tuple(sorted(({**CFG, **(cfg or {})}).items()))
    if key not in _CACHE:
        _CACHE[key] = _build_program(cfg)
    return _CACHE[key]


def make_in_maps(x, params):
    B = x.shape[0]
    per = B // N_CORES
    xr = np.ascontiguousarray(x.reshape(B, CIN, N), dtype=np.float32)
    return [
        {"xs": xr[c * per:(c + 1) * per], **params}
        for c in range(N_CORES)
    ]


def kernel(x, w1, bn1_g, bn1_b, bn1_m, bn1_v, wq, bq, wk, bk, wv, bv,
           rel_h, rel_w, w2, bn2_g, bn2_b, bn2_m, bn2_v):
    from concourse.bass_utils import run_bass_kernel_spmd

    nc = get_program()
    params = _prepare_params(w1, bn1_g, bn1_b, bn1_m, bn1_v, wq, bq, wk, bk,
                             wv, bv, rel_h, rel_w, w2, bn2_g, bn2_b, bn2_m,
                             bn2_v)
    in_maps = make_in_maps(x, params)
    res = run_bass_kernel_spmd(nc, in_maps, core_ids=list(range(N_CORES)))
    out = np.concatenate([res.results[c]["ys"] for c in range(N_CORES)], axis=0)
    return np.ascontiguousarray(out.reshape(32, CIN, 32, 32), dtype=np.float32)
